# revision 2
# baseline (speedup 1.0000x reference)
"""CrossTransformerBlock3D Trainium2 kernel, v2.

Shards D (32) into 8 slabs of 4 across 8 NeuronCores; each core runs the
full block on its slab (256 windows of 64 tokens, groups of 8 windows =
512 tokens) with no collectives.

v2 structure (vs v1):
  - attention in window-PAIR blocks: score tiles [128 keys-of-pair, 128
    queries-of-pair] per (u, head); cross-window quadrants are masked by a
    -30 additive bias so exp() zeroes them; v needs no slot duplication
    (the token-major [128, 192] v tile is the AV lhsT directly).
  - relative-position bias + mask is PRELOADED into PSUM via a bias^T @ I
    matmul (start=True), so no DVE bias-add.
  - phase-major emission in batches of BATCH groups: P1 (LN1+QKV),
    P2 (attention), P3 (proj+residual+LN2), P4 (MLP+residual) - ACT
    switches activation tables 4x per batch instead of ~8x per group.
  - merged DMAs: one load for x, one for y, one store per group.
  - LN applies on the (otherwise idle) Pool engine; PSUM evictions split
    between ACT and DVE; LN sqrt batched to one ACT op per phase.
All matmuls bf16 with fp32 PSUM accumulation.
"""

import math
import numpy as np
import ml_dtypes

import concourse.bass as bass
import concourse.tile as tile
from concourse import bacc, mybir
from concourse.bass_utils import run_bass_kernel_spmd

F32 = mybir.dt.float32
BF16 = mybir.dt.bfloat16
AF = mybir.ActivationFunctionType
ALU = mybir.AluOpType

# Problem shape (hardcoded per contract)
B, D, H, W, C = 1, 32, 64, 64, 192
NH, HD = 6, 32
SCALE = HD ** -0.5
N_CORES = 8
DS = D // N_CORES            # 4 depth per core = one window depth
NWH, NWW = H // 4, W // 4    # 16 x 16 windows per core
N_WIN = NWH * NWW            # 256 windows/core
GROUP_WIN = 8                # windows per group (512 tokens)
N_GROUPS = N_WIN // GROUP_WIN  # 32
BATCH = 4                    # groups per phase-major batch
FFN = 4 * C                  # 768
MASK_NEG = -30.0


def _rel_index():
    ws = (4, 4, 4)
    coords = np.stack(np.meshgrid(np.arange(ws[0]), np.arange(ws[1]), np.arange(ws[2]), indexing='ij'))
    cf = coords.reshape(3, -1)
    rel = (cf[:, :, None] - cf[:, None, :]).transpose(1, 2, 0).copy()
    rel[:, :, 0] += ws[0] - 1
    rel[:, :, 1] += ws[1] - 1
    rel[:, :, 2] += ws[2] - 1
    rel[:, :, 0] *= (2 * ws[1] - 1) * (2 * ws[2] - 1)
    rel[:, :, 1] *= 2 * ws[2] - 1
    return rel.sum(-1)


def bf16(a):
    return np.asarray(a, np.float32).astype(ml_dtypes.bfloat16)


def win_permute(slab):
    """[DS,H,W,C] -> [N_WIN*64, C] in (wh, ww, d, i, j) token order."""
    t = slab.reshape(DS, NWH, 4, NWW, 4, C).transpose(1, 3, 0, 2, 4, 5)
    return np.ascontiguousarray(t.reshape(N_WIN * 64, C))


def win_unpermute(flat):
    """[N_WIN*64, C] -> [DS,H,W,C]."""
    t = flat.reshape(NWH, NWW, DS, 4, 4, C).transpose(2, 0, 3, 1, 4, 5)
    return np.ascontiguousarray(t.reshape(DS, H, W, C))


def build_program(weights_np=None):
    nc = bacc.Bacc("TRN2", target_bir_lowering=False, debug=False)

    xs = nc.dram_tensor("xs", [N_WIN * 64, C], F32, kind="ExternalInput").ap()
    ys = nc.dram_tensor("ys", [N_WIN * 64, C], F32, kind="ExternalInput").ap()
    wq = nc.dram_tensor("wq", [C, C], BF16, kind="ExternalInput").ap()
    wk = nc.dram_tensor("wk", [C, C], BF16, kind="ExternalInput").ap()
    wv = nc.dram_tensor("wv", [C, C], BF16, kind="ExternalInput").ap()
    wp = nc.dram_tensor("wp", [C, C], BF16, kind="ExternalInput").ap()
    w1 = nc.dram_tensor("w1", [C, FFN], BF16, kind="ExternalInput").ap()
    w2 = nc.dram_tensor("w2", [FFN, C], BF16, kind="ExternalInput").ap()
    # bias6T[q, h, k] = bias^T for head h in query-major order (used as
    # lhsT of the PSUM-preload matmul), incl. the -30 cross-window mask.
    bias6T = nc.dram_tensor("bias6T", [128, NH, 128], BF16, kind="ExternalInput").ap()
    # eh[k, h, j] = (j == h): lhsT slice eh[:, h, :] routes head h's rowsum
    # into row h of the [NH, 512] accumulator, zeros elsewhere.
    eh = nc.dram_tensor("eh", [128, NH, NH], BF16, kind="ExternalInput").ap()
    ident = nc.dram_tensor("ident", [128, 128], BF16, kind="ExternalInput").ap()
    out = nc.dram_tensor("out", [N_WIN * 64, C], F32, kind="ExternalOutput").ap()

    with tile.TileContext(nc) as tc:
        kernel_body(tc, xs, ys, wq, wk, wv, wp, w1, w2, bias6T, eh, ident, out)
    nc.compile()
    return nc


def kernel_body(tc, xs, ys, wq, wk, wv, wp, w1, w2, bias6T, eh, ident, out):
    nc = tc.nc
    ctx_pools = []

    def pool(name, bufs, space="SBUF"):
        p = tc.tile_pool(name=name, bufs=bufs, space=space)
        ctx_pools.append(p)
        return p.__enter__()

    singles = pool("singles", 1)
    sb2 = pool("sb2", 2)       # transient within one phase
    sb6 = pool("sb6", 6)       # live across one phase boundary
    sb3 = pool("sb3", 3)
    # PSUM: 8 banks of [128, 512]xf32; bank-per-buffer granularity.
    ps_tp = pool("ps_tp", 2, space="PSUM")    # transpose staging (2 banks)
    ps_mm = pool("ps_mm", 2, space="PSUM")    # linears + rowsums (2 banks)
    ps_sc = pool("ps_sc", 2, space="PSUM")    # scores, 3 heads/tile (2 banks)
    ps_ao = pool("ps_ao", 1, space="PSUM")    # attention out (2 banks)

    def load_const(name, src_ap, shape, dtype):
        t = singles.tile(shape, dtype, tag=name)
        nc.sync.dma_start(out=t, in_=src_ap)
        return t

    wq_hi = load_const("wq_hi", wq[0:128, :], [128, C], BF16)
    wq_lo = load_const("wq_lo", wq[128:192, :], [64, C], BF16)
    wk_hi = load_const("wk_hi", wk[0:128, :], [128, C], BF16)
    wk_lo = load_const("wk_lo", wk[128:192, :], [64, C], BF16)
    wv_hi = load_const("wv_hi", wv[0:128, :], [128, C], BF16)
    wv_lo = load_const("wv_lo", wv[128:192, :], [64, C], BF16)
    wp_hi = load_const("wp_hi", wp[0:128, :], [128, C], BF16)
    wp_lo = load_const("wp_lo", wp[128:192, :], [64, C], BF16)
    w1_hi = load_const("w1_hi", w1[0:128, :], [128, FFN], BF16)
    w1_lo = load_const("w1_lo", w1[128:192, :], [64, FFN], BF16)
    w2_sb = load_const("w2_sb", w2.rearrange("(k p) c -> p k c", p=128), [128, 6, C], BF16)
    bias6T_sb = load_const("bias6T_sb", bias6T, [128, NH, 128], BF16)
    eh_sb = load_const("eh_sb", eh, [128, NH, NH], BF16)
    ident_sb = load_const("ident_sb", ident, [128, 128], BF16)
    eps_sb = singles.tile([128, 1], F32, tag="eps")
    nc.vector.memset(eps_sb, 1e-5)

    # ---------------- phase bodies ----------------
    # state[g] holds live tiles of group g between phases
    state = {}

    def ln_stats(src_f32, mv_all, idx, u_pool):
        """bn_stats/aggr of [128, C] -> mv_all[:, :, idx] (mean, var)."""
        st = u_pool.tile([128, 6], F32, tag="ln_st")
        nc.vector.bn_stats(out=st, in_=src_f32)
        nc.vector.bn_aggr(out=mv_all[:, :, idx], in_=st)

    def ln_finalize(mv_all, n):
        """var -> 1/sqrt(var+eps) in-place for n stat columns."""
        nc.scalar.activation(out=mv_all[:, 1, 0:n], in_=mv_all[:, 1, 0:n],
                             func=AF.Sqrt, bias=eps_sb, scale=1.0)
        nc.vector.reciprocal(out=mv_all[:, 1, 0:n], in_=mv_all[:, 1, 0:n])

    def evict(engine, out, in_):
        """PSUM -> SBUF copy on the chosen engine (ACT or DVE)."""
        if engine is nc.scalar:
            nc.scalar.activation(out=out, in_=in_, func=AF.Copy)
        else:
            engine.tensor_copy(out=out, in_=in_)

    def transpose_pair(dst_hi, dst_lo, src_bf16, u, evict_engine):
        """[128, 192] token-major -> cols 128u of feature-major dst pair."""
        t = ps_tp.tile([128, 256], BF16, tag="tp")
        nc.tensor.transpose(t[:, 0:128], src_bf16[:, 0:128], ident_sb)
        nc.tensor.transpose(t[0:64, 128:256], src_bf16[:, 128:192], ident_sb)
        evict(evict_engine, dst_hi[:, 128 * u:128 * u + 128], t[:, 0:128])
        evict(evict_engine, dst_lo[:, 128 * u:128 * u + 128], t[0:64, 128:256])

    def linear_fm(dst_hi, dst_lo, lhs_hi, lhs_lo, rhs_hi, rhs_lo,
                  evict_hi, evict_lo):
        """dst[o, n] = sum_c lhs[c, o] * rhs[c, n]; evict via given engines."""
        p_hi = ps_mm.tile([128, 512], F32, tag="mm")
        nc.tensor.matmul(p_hi, lhs_hi[:, 0:128], rhs_hi, start=True, stop=False)
        nc.tensor.matmul(p_hi, lhs_lo[:, 0:128], rhs_lo, start=False, stop=True)
        evict(evict_hi, dst_hi, p_hi)
        p_lo = ps_mm.tile([128, 512], F32, tag="mm")
        p_lo = p_lo[0:64, :]
        nc.tensor.matmul(p_lo, lhs_hi[:, 128:192], rhs_hi, start=True, stop=False)
        nc.tensor.matmul(p_lo, lhs_lo[:, 128:192], rhs_lo, start=False, stop=True)
        evict(evict_lo, dst_lo, p_lo)

    def phase1(g):
        """Load x/y, LN1, transpose to feature-major, q/k/v."""
        x_keep = sb6.tile([128, 4, C], F32, tag="x_keep")
        nc.sync.dma_start(
            out=x_keep,
            in_=xs[g * 512:(g + 1) * 512, :].rearrange("(u p) c -> p u c", u=4))
        y_keep = sb2.tile([128, 4, C], F32, tag="y_keep")
        nc.sync.dma_start(
            out=y_keep,
            in_=ys[g * 512:(g + 1) * 512, :].rearrange("(u p) c -> p u c", u=4))

        mv_x = sb2.tile([128, 2, 8], F32, tag="mv_x")
        for u in range(4):
            ln_stats(x_keep[:, u, :], mv_x, u, sb3)
            ln_stats(y_keep[:, u, :], mv_x, 4 + u, sb3)
        ln_finalize(mv_x, 8)

        xnT_hi = sb2.tile([128, 512], BF16, tag="xnT_hi")
        xnT_lo = sb2.tile([64, 512], BF16, tag="xnT_lo")
        ynT_hi = sb2.tile([128, 512], BF16, tag="ynT_hi")
        ynT_lo = sb2.tile([64, 512], BF16, tag="ynT_lo")
        for u in range(4):
            xn_t = sb3.tile([128, C], BF16, tag="xn_t")
            nc.gpsimd.tensor_scalar(out=xn_t, in0=x_keep[:, u, :],
                                    scalar1=mv_x[:, 0, u:u + 1],
                                    scalar2=mv_x[:, 1, u:u + 1],
                                    op0=ALU.subtract, op1=ALU.mult)
            transpose_pair(xnT_hi, xnT_lo, xn_t, u, nc.scalar)
            yn_t = sb3.tile([128, C], BF16, tag="yn_t")
            nc.gpsimd.tensor_scalar(out=yn_t, in0=y_keep[:, u, :],
                                    scalar1=mv_x[:, 0, 4 + u:5 + u],
                                    scalar2=mv_x[:, 1, 4 + u:5 + u],
                                    op0=ALU.subtract, op1=ALU.mult)
            transpose_pair(ynT_hi, ynT_lo, yn_t, u, nc.vector)

        q_hi = sb6.tile([128, 512], BF16, tag="q_hi")
        q_lo = sb6.tile([64, 512], BF16, tag="q_lo")
        linear_fm(q_hi, q_lo, wq_hi, wq_lo, ynT_hi, ynT_lo, nc.scalar, nc.scalar)
        k_hi = sb6.tile([128, 512], BF16, tag="k_hi")
        k_lo = sb6.tile([64, 512], BF16, tag="k_lo")
        linear_fm(k_hi, k_lo, wk_hi, wk_lo, xnT_hi, xnT_lo, nc.scalar, nc.vector)

        v_sb = sb6.tile([128, 4, C], BF16, tag="v_sb")
        for u in range(4):
            v_ps = ps_mm.tile([128, 512], F32, tag="mm")
            v_ps = v_ps[:, 0:C]
            nc.tensor.matmul(v_ps, xnT_hi[:, 128 * u:128 * u + 128], wv_hi,
                             start=True, stop=False)
            nc.tensor.matmul(v_ps, xnT_lo[:, 128 * u:128 * u + 128], wv_lo,
                             start=False, stop=True)
            nc.vector.tensor_copy(out=v_sb[:, u, :], in_=v_ps)
        state[g] = dict(x_keep=x_keep, q_hi=q_hi, q_lo=q_lo,
                        k_hi=k_hi, k_lo=k_lo, v_sb=v_sb)

    def phase2(g):
        """Pair-block attention: scores+bias, exp, rowsums, AV, normalize."""
        st = state[g]
        q_hi, q_lo = st["q_hi"], st["q_lo"]
        k_hi, k_lo = st["k_hi"], st["k_lo"]
        v_sb = st["v_sb"]

        attn = sb2.tile([128, 4, NH, 128], BF16, tag="attn")
        for u in range(4):
            scA = ps_sc.tile([128, 3, 128], F32, tag="sc", name=f"scA_{g}_{u}")
            scB = ps_sc.tile([128, 3, 128], F32, tag="sc", name=f"scB_{g}_{u}")
            scs = [scA, scB]
            for h in range(NH):
                if h < 4:
                    k_sl, q_sl, off = k_hi, q_hi, 32 * h
                else:
                    k_sl, q_sl, off = k_lo, q_lo, 32 * (h - 4)
                sc_h = scs[h // 3][:, h % 3, :]
                nc.tensor.matmul(sc_h, bias6T_sb[:, h, :], ident_sb,
                                 start=True, stop=False)
                nc.tensor.matmul(
                    sc_h,
                    k_sl[off:off + 32, 128 * u:128 * u + 128],
                    q_sl[off:off + 32, 128 * u:128 * u + 128],
                    start=False, stop=True,
                    tile_position=(off, 0),
                )
            nc.scalar.activation(out=attn[:, u, 0:3, :], in_=scs[0], func=AF.Exp)
            nc.scalar.activation(out=attn[:, u, 3:6, :], in_=scs[1], func=AF.Exp)

        # rowsums over keys: r6[h, (u, q)] via indicator matmuls
        # (borrows an mm buffer; mm pool is otherwise idle during P2)
        r6_ps = ps_mm.tile([128, 512], F32, tag="mm")
        r6_ps = r6_ps[0:NH, :]
        for h in range(NH):
            nc.tensor.matmul(r6_ps, eh_sb[:, h, :], attn[:, :, h, :],
                             start=(h == 0), stop=(h == NH - 1))
        r6_sb = sb2.tile([NH, 512], BF16, tag="r6_sb")
        with nc.allow_low_precision(reason="softmax 1/sum in bf16"):
            nc.vector.reciprocal(out=r6_sb, in_=r6_ps)
        rbc_hi = sb2.tile([128, 512], BF16, tag="rbc_hi")
        nc.sync.dma_start(out=rbc_hi,
                          in_=r6_sb[0:4, :].unsqueeze(1).broadcast_to([4, 32, 512]))
        rbc_lo = sb2.tile([64, 512], BF16, tag="rbc_lo")
        nc.sync.dma_start(out=rbc_lo,
                          in_=r6_sb[4:6, :].unsqueeze(1).broadcast_to([2, 32, 512]))

        ao = ps_ao.tile([128, 1024], F32, tag="ao")
        ao_hi = ao[:, 0:512]
        ao_lo = ao[0:64, 512:1024]
        for u in range(4):
            for h in range(NH):
                if h < 4:
                    dst, off = ao_hi, 32 * h
                else:
                    dst, off = ao_lo, 32 * (h - 4)
                nc.tensor.matmul(
                    dst[off:off + 32, 128 * u:128 * u + 128],
                    v_sb[:, u, 32 * h:32 * h + 32],
                    attn[:, u, h, :],
                    start=True, stop=True,
                    tile_position=(0, off),
                )
        aoT_hi = sb6.tile([128, 512], BF16, tag="aoT_hi")
        nc.vector.tensor_tensor(out=aoT_hi, in0=ao_hi, in1=rbc_hi, op=ALU.mult)
        aoT_lo = sb6.tile([64, 512], BF16, tag="aoT_lo")
        nc.vector.tensor_tensor(out=aoT_lo, in0=ao_lo, in1=rbc_lo, op=ALU.mult)
        st["aoT_hi"], st["aoT_lo"] = aoT_hi, aoT_lo
        for k in ("q_hi", "q_lo", "k_hi", "k_lo", "v_sb"):
            del st[k]

    def phase3(g):
        """proj, residual, LN2, transpose back to feature-major."""
        st = state[g]
        poT_hi = sb2.tile([128, 512], BF16, tag="poT_hi")
        poT_lo = sb2.tile([64, 512], BF16, tag="poT_lo")
        linear_fm(poT_hi, poT_lo, wp_hi, wp_lo, st["aoT_hi"], st["aoT_lo"],
                  nc.scalar, nc.scalar)

        x_keep = st["x_keep"]
        x2 = sb6.tile([128, 4, C], F32, tag="x2")
        mv2 = sb2.tile([128, 2, 4], F32, tag="mv2")
        for u in range(4):
            t = ps_tp.tile([128, 256], BF16, tag="tp")
            nc.tensor.transpose(t[:, 0:128], poT_hi[:, 128 * u:128 * u + 128],
                                ident_sb)
            nc.tensor.transpose(t[:, 128:192], poT_lo[:, 128 * u:128 * u + 128],
                                ident_sb[0:64, 0:64])
            nc.vector.tensor_tensor(out=x2[:, u, 0:128], in0=t[:, 0:128],
                                    in1=x_keep[:, u, 0:128], op=ALU.add)
            nc.vector.tensor_tensor(out=x2[:, u, 128:192], in0=t[:, 128:192],
                                    in1=x_keep[:, u, 128:192], op=ALU.add)
            ln_stats(x2[:, u, :], mv2, u, sb3)
        ln_finalize(mv2, 4)

        xn2T_hi = sb6.tile([128, 512], BF16, tag="xn2T_hi")
        xn2T_lo = sb6.tile([64, 512], BF16, tag="xn2T_lo")
        for u in range(4):
            xn2_t = sb3.tile([128, C], BF16, tag="xn2_t")
            nc.gpsimd.tensor_scalar(out=xn2_t, in0=x2[:, u, :],
                                    scalar1=mv2[:, 0, u:u + 1],
                                    scalar2=mv2[:, 1, u:u + 1],
                                    op0=ALU.subtract, op1=ALU.mult)
            transpose_pair(xn2T_hi, xn2T_lo, xn2_t, u, nc.vector)
        st["x2"] = x2
        st["xn2T_hi"], st["xn2T_lo"] = xn2T_hi, xn2T_lo
        for k in ("x_keep", "aoT_hi", "aoT_lo"):
            del st[k]

    def phase4(g):
        """MLP with fused GELU, final residual, store."""
        st = state[g]
        xn2T_hi, xn2T_lo = st["xn2T_hi"], st["xn2T_lo"]
        hT = sb2.tile([128, 6, 512], BF16, tag="hT")
        for m in range(6):
            h_ps = ps_mm.tile([128, 512], F32, tag="mm")
            nc.tensor.matmul(h_ps, w1_hi[:, 128 * m:128 * m + 128], xn2T_hi,
                             start=True, stop=False)
            nc.tensor.matmul(h_ps, w1_lo[:, 128 * m:128 * m + 128], xn2T_lo,
                             start=False, stop=True)
            nc.scalar.activation(out=hT[:, m, :], in_=h_ps, func=AF.Gelu)

        po2_hi = ps_mm.tile([128, 512], F32, tag="mm")
        po2_lo = ps_mm.tile([128, 512], F32, tag="mm")
        po2_lo = po2_lo[0:64, :]
        for kc in range(6):
            nc.tensor.matmul(po2_hi, w2_sb[:, kc, 0:128], hT[:, kc, :],
                             start=(kc == 0), stop=(kc == 5))
        for kc in range(6):
            nc.tensor.matmul(po2_lo, w2_sb[:, kc, 128:192], hT[:, kc, :],
                             start=(kc == 0), stop=(kc == 5))
        po2T_hi = sb2.tile([128, 512], BF16, tag="po2T_hi")
        nc.scalar.activation(out=po2T_hi, in_=po2_hi, func=AF.Copy)
        po2T_lo = sb2.tile([64, 512], BF16, tag="po2T_lo")
        nc.scalar.activation(out=po2T_lo, in_=po2_lo, func=AF.Copy)

        x2 = st["x2"]
        o_t = sb2.tile([128, 4, C], F32, tag="o_t")
        for u in range(4):
            t = ps_tp.tile([128, 256], BF16, tag="tp")
            nc.tensor.transpose(t[:, 0:128], po2T_hi[:, 128 * u:128 * u + 128],
                                ident_sb)
            nc.tensor.transpose(t[:, 128:192], po2T_lo[:, 128 * u:128 * u + 128],
                                ident_sb[0:64, 0:64])
            nc.vector.tensor_tensor(out=o_t[:, u, 0:128], in0=t[:, 0:128],
                                    in1=x2[:, u, 0:128], op=ALU.add)
            nc.vector.tensor_tensor(out=o_t[:, u, 128:192], in0=t[:, 128:192],
                                    in1=x2[:, u, 128:192], op=ALU.add)
        nc.sync.dma_start(
            out=out[g * 512:(g + 1) * 512, :].rearrange("(u p) c -> p u c", u=4),
            in_=o_t)
        del state[g]

    for b0 in range(0, N_GROUPS, BATCH):
        gs = range(b0, b0 + BATCH)
        for g in gs:
            phase1(g)
        for g in gs:
            phase2(g)
        for g in gs:
            phase3(g)
        for g in gs:
            phase4(g)

    for p in reversed(ctx_pools):
        p.__exit__(None, None, None)


def prep_inputs(inputs):
    """Host-side prep: fold norms/scales into weights, build constants."""
    f32 = lambda a: np.ascontiguousarray(np.asarray(a, np.float32))
    x, y = f32(inputs['x']), f32(inputs['y'])
    qkv_w, qkv_b = f32(inputs['qkv_w']), f32(inputs['qkv_b'])
    g1, b1n = f32(inputs['norm1_g']), f32(inputs['norm1_b'])
    g2, b2n = f32(inputs['norm2_g']), f32(inputs['norm2_b'])

    wq_eff = g1[:, None] * qkv_w[:, 0:C] * SCALE
    wk_eff = g1[:, None] * qkv_w[:, C:2 * C]
    wv_eff = g1[:, None] * qkv_w[:, 2 * C:]
    bq = b1n @ qkv_w[:, 0:C] * SCALE + qkv_b[0:C] * SCALE
    bk = b1n @ qkv_w[:, C:2 * C] + qkv_b[C:2 * C]
    bv = b1n @ qkv_w[:, 2 * C:] + qkv_b[2 * C:]
    w1_eff = g2[:, None] * f32(inputs['fc1_w'])
    b1_eff = b2n @ f32(inputs['fc1_w']) + f32(inputs['fc1_b'])
    assert not (np.any(bq) or np.any(bk) or np.any(bv) or np.any(b1_eff) or
                np.any(f32(inputs['proj_b'])) or np.any(f32(inputs['fc2_b']))), \
        "nonzero biases not folded in this build"

    rel = _rel_index()
    rpb = f32(inputs['rpb_table'])
    bias_full = rpb[rel]                     # [n, m, NH] (query n, key m)
    # bias6T[q, h, k]: pair-block layout, query-major (lhsT of preload
    # matmul out[k, q] = sum_m lhsT[m=q', k'] I[m, q]).  Diagonal window
    # quadrants get the rpb bias; off-diagonal quadrants get MASK_NEG.
    bias6T = np.full((128, NH, 128), MASK_NEG, np.float32)
    for h in range(NH):
        for blk in range(2):
            s = 64 * blk
            # out[k, q] for k,q in same window: bias_full[q, k, h]
            bias6T[s:s + 64, h, s:s + 64] = bias_full[:, :, h]
    # eh[k, h, j] = (j == h): all 128 pair keys summed into row h (masked
    # entries contribute exp(-30)~1e-13, negligible)
    eh_m = np.zeros((128, NH, NH), np.float32)
    for h in range(NH):
        eh_m[:, h, h] = 1.0
    ident = np.eye(128, dtype=np.float32)

    shared = {
        'wq': bf16(wq_eff), 'wk': bf16(wk_eff), 'wv': bf16(wv_eff),
        'wp': bf16(inputs['proj_w']), 'w1': bf16(w1_eff),
        'w2': bf16(inputs['fc2_w']),
        'bias6T': bf16(bias6T), 'ident': bf16(ident), 'eh': bf16(eh_m),
    }

    in_maps = []
    for i in range(N_CORES):
        m = dict(shared)
        m['xs'] = win_permute(x[0, i * DS:(i + 1) * DS])
        m['ys'] = win_permute(y[0, i * DS:(i + 1) * DS])
        in_maps.append(m)
    return in_maps


_CACHED_NC = None


def get_program(in_maps=None):
    global _CACHED_NC
    if _CACHED_NC is None:
        _CACHED_NC = build_program()
    return _CACHED_NC


def kernel(**inputs):
    in_maps = prep_inputs(inputs)
    nc = get_program(in_maps)
    res = run_bass_kernel_spmd(nc, in_maps, list(range(N_CORES)))
    outs = [win_unpermute(res.results[i]["out"]) for i in range(N_CORES)]
    full = np.concatenate([o[None] for o in outs], axis=0)  # [8, DS, H, W, C]
    full = full.reshape(1, D, H, W, C).astype(np.float32)
    return full
